# revision 32
# baseline (speedup 1.0000x reference)
"""Trainium2 Bass kernel for GQA attention block (nn_Attention_6219112644965).

Reference computation (per batch b):
  q = rope(rmsnorm(x @ Wq, q_gamma), cos, sin)   # 16 heads x 128
  k = rope(rmsnorm(x @ Wk, k_gamma), cos, sin)   # 8 kv heads x 128
  v = x @ Wv
  o = softmax(q k^T / sqrt(128)) v               # GQA: q head h uses kv head h//2
  y = o @ Wo
Sharding: 8 cores = 4 batches x 2 head-groups; each core emits a partial
y (its 8 heads' contribution); host sums the two partials per batch.

Everything is computed head-dim-major (q^T, k^T: [HD=128 partitions, T free])
so no transposes are needed anywhere: projections use the weight slab as
stationary lhsT over x^T, scores/out accumulate transposed, and the final
y = (o^T)^T Wo.  Partition-axis reductions (rmsnorm sum of squares, softmax
denominator) are ones-matmuls ([1,512] rows, ~0.3us each) -- measured much
faster end-to-end than GpSimd partition_all_reduce (~4us + library reloads).

Schedule (the point of this version): one merged front phase emits
K-projections, V, then software-pipelines attention chunk 0 of head h with the
projection of head h+2 at ~1us instruction granularity, so ScalarE's exp
stream (the attention pacer: ~0.56us/512-tile vs ~0.43us of PE work) hides
under projection matmuls instead of stalling the in-order PE.  xt is loaded
column-chunk-major so the first projection chain starts ~6us in (one 2.1MB
chunk) instead of waiting for the full 8.4MB.  The back phase interleaves
attention chunk 1 with the chunk-0 out-projection rows, and the tail
out-projection gets its own 4-buffer PSUM pool so PSUM->SBUF copies never gate
the matmul stream.  softmax max-subtraction is skipped (scores are O(5) for
rmsnorm-ed q,k; fp32 exp is exact there); rmsnorm scale and rope tables are
host-folded (head-dim permuted so the rope pair shuffle is a 64-partition
half swap; gamma and the rope sign live in the cos/sin tables; the rmsnorm
row scale commutes with rope and is applied once at the end).
"""
import sys

sys.path.insert(0, "/opt/trn_rl_repo")

from contextlib import ExitStack

import ml_dtypes
import numpy as np

import bass_rust
import concourse.bass as bass
import concourse.mybir as mybir
import concourse.tile as tile
from concourse import bacc, bass_isa, hw_specs
from concourse.bass_utils import run_bass_kernel_spmd

F32 = mybir.dt.float32
BF16 = mybir.dt.bfloat16
AF = mybir.ActivationFunctionType
RADD = bass_isa.ReduceOp.add

T = 2048          # sequence length
D = 2048          # model dim
HD = 128          # head dim
NQH = 8           # q heads per core
NKV = 4           # kv heads per core
ND = D // 128     # 16 d-tiles
NTT = T // 128    # 16 t-tiles
NCH = T // 512    # 4 column chunks
EPS = 1e-6

_CACHE = {}
LAST_RESULTS = None


class _Bacc(bacc.Bacc):
    """Bacc with Exp pinned to the natural_log_exp_and_others ACT table set.

    The default static func->set assignment maps Exp to `exp_and_others`
    and Ln to `natural_log_exp_and_others`; a kernel alternating Ln and Exp
    then reloads the ACT tables (~2.7us) on every transition.  Hiding `exp`
    from the other sets makes both resolve to the shared set, so the table
    is loaded once for the whole kernel.
    """

    def insert_act_table_loads(self):
        has_activation = any(
            isinstance(i, mybir.InstActivation)
            for b in self.main_func.blocks
            for i in b.instructions
        )
        if not has_activation:
            return
        tables = []
        for name, funcs in hw_specs.get_activation_tables(self.m.arch).items():
            if name != "natural_log_exp_and_others":
                funcs = funcs - {AF.Exp}
            tables.append((name, funcs))
        bass_rust.insert_act_table_loads(self, tables)


def _drain(g):
    for _ in g:
        pass


def _roundrobin(gens, steps_per_turn):
    """Advance each generator steps_per_turn steps in rotation until all are
    exhausted.  Used to let several K-projection heads consume the chunk-major
    xt stream at its DMA arrival rate."""
    alive = list(gens)
    while alive:
        for g in list(alive):
            for _ in range(steps_per_turn):
                try:
                    next(g)
                except StopIteration:
                    alive.remove(g)
                    break


def _interleave(main, filler, fill_per_main):
    """Drive `main`; after each of its steps emit ~fill_per_main steps of
    `filler`.  Threads PE-dense filler work between exp-gated attention steps
    so the in-order PE stream never waits on ScalarE."""
    credit = 0.0
    alive = filler is not None
    for _ in main:
        credit += fill_per_main
        while credit >= 1.0 and alive:
            try:
                next(filler)
            except StopIteration:
                alive = False
            credit -= 1.0
    while alive:
        try:
            next(filler)
        except StopIteration:
            alive = False


def build_module():
    """Build the per-core Bass program (identical on all 8 cores)."""
    nc = _Bacc("TRN2", target_bir_lowering=False, debug=False)

    # ---- DRAM I/O (host-packed so every DMA is contiguous) ----
    xt_d = nc.dram_tensor("xt", [NCH, 128, ND, 512], BF16, kind="ExternalInput")
    wq_d = nc.dram_tensor("wq", [NQH, 128, ND, HD], BF16, kind="ExternalInput")
    wk_d = nc.dram_tensor("wk", [NKV, 128, ND, HD], BF16, kind="ExternalInput")
    wv_d = nc.dram_tensor("wv", [128, ND, NKV * HD], BF16, kind="ExternalInput")
    wo_d = nc.dram_tensor("wo", [128, NQH, D], BF16, kind="ExternalInput")
    cosq_d = nc.dram_tensor("cosq", [128, T], BF16, kind="ExternalInput")
    sinq_d = nc.dram_tensor("sinq", [128, T], BF16, kind="ExternalInput")
    cosk_d = nc.dram_tensor("cosk", [128, T], BF16, kind="ExternalInput")
    sink_d = nc.dram_tensor("sink", [128, T], BF16, kind="ExternalInput")
    y_d = nc.dram_tensor("y", [T, D], F32, kind="ExternalOutput")

    with tile.TileContext(nc) as tc, ExitStack() as top:
        persist = top.enter_context(tc.tile_pool(name="persist", bufs=1))
        qT = persist.tile([128, NQH, T], BF16, tag="qT")     # q^T, rope+norm done
        kT = persist.tile([128, NKV, T], BF16, tag="kT")     # k^T, rope+norm done
        v_sb = persist.tile([128, NTT, NKV * HD], BF16, tag="v")  # v natural
        oT0 = persist.tile([128, NQH, 1024], BF16, tag="oT0")  # attn out, tq 0:1024
        ones_col = persist.tile([128, 1], BF16, tag="ones")
        nc.vector.memset(ones_col, 1.0)
        zero128 = persist.tile([128, 1], F32, tag="zero128")
        nc.vector.memset(zero128, 0.0)
        epsq = persist.tile([128, 1], F32, tag="epsq")
        nc.vector.memset(epsq, float(HD * EPS))
        epsk = persist.tile([128, 1], F32, tag="epsk")
        nc.vector.memset(epsk, float(EPS))

        # ================= phase A: projections + attention chunk 0 =========
        with ExitStack() as phA:
            xtp = phA.enter_context(tc.tile_pool(name="xtp", bufs=1))
            xt = xtp.tile([128, ND, T], BF16, tag="xt")
            pw = phA.enter_context(tc.tile_pool(name="pw", bufs=2))
            pwrow = phA.enter_context(tc.tile_pool(name="pwrow", bufs=1))
            ps_mm = phA.enter_context(tc.tile_pool(name="ps_mm", bufs=2, space="PSUM"))
            ps_ssq = phA.enter_context(tc.tile_pool(name="ps_ssq", bufs=1, space="PSUM"))
            wvp = phA.enter_context(tc.tile_pool(name="wvp", bufs=1))
            wv_sb = wvp.tile([128, ND, NKV * HD], BF16, tag="wv")
            # ktab scoped: its 24K frees right after K for the q-table/slab
            # pools (whose DMAs then fire at ~112us, well before Q0 needs
            # them, instead of waiting for a later pool-close event)
            ktab_cm = tc.tile_pool(name="ktab", bufs=1)
            ktab = ktab_cm.__enter__()
            cosk = ktab.tile([128, T], BF16, tag="cosk")
            sink = ktab.tile([128, T], BF16, tag="sink")
            wsl_k = ktab.tile([NKV, 128, ND, HD], BF16, tag="wslk")

            # DMA order IS the arrival order (one striped queue): k-head
            # slabs and the first xt chunk lead so the K matmul round-robin
            # starts ~4us in and then tracks the xt chunk arrivals.
            nc.sync.dma_start(out=wsl_k[0], in_=wk_d[0])
            nc.sync.dma_start(out=wsl_k[1], in_=wk_d[1])
            nc.sync.dma_start(out=xt[:, 0:8, 0:512], in_=xt_d[0][:, 0:8, :])
            nc.sync.dma_start(out=xt[:, 8:ND, 0:512], in_=xt_d[0][:, 8:ND, :])
            nc.sync.dma_start(out=cosk[:], in_=cosk_d[:])
            nc.sync.dma_start(out=sink[:], in_=sink_d[:])
            nc.sync.dma_start(out=wsl_k[2], in_=wk_d[2])
            nc.sync.dma_start(out=wsl_k[3], in_=wk_d[3])
            for ch in range(1, NCH):
                nc.sync.dma_start(out=xt[:, :, ch * 512:(ch + 1) * 512],
                                  in_=xt_d[ch])
            nc.sync.dma_start(out=wv_sb[:], in_=wv_d[:])

            def qk_proj_steps(h, w_dram, out_T, cos_t, sin_t, is_q, wsl=None):
                """One head's projection + rmsnorm + rope (3 yields/chunk).

                Per 512-chunk: 16 chained matmuls, then sum-of-squares over
                the head dim via ones-matmul, rsqrt via ln/exp on ScalarE
                (one ACT table set), GpSimd broadcast, rope muls + final
                normalize on DVE."""
                if wsl is None:
                    wsl = wslab_p.tile([128, ND, HD], BF16, tag="wsl")
                    nc.sync.dma_start(out=wsl[:], in_=w_dram[h])
                for c in range(NCH):
                    cs = slice(c * 512, (c + 1) * 512)
                    ps = ps_mm.tile([128, 512], F32, tag="mm")
                    for d in range(ND):
                        nc.tensor.matmul(ps, wsl[:, d, :], xt[:, d, cs],
                                         start=(d == 0), stop=(d == ND - 1))
                        if d == 7:
                            yield
                    raw = pw.tile([128, 512], BF16, tag="raw")
                    # alternate the copy between DVE and ScalarE: in the
                    # interleaved window both run near 85% -- pinning it to
                    # either one makes that engine the pacer
                    if c % 2 == 0:
                        nc.vector.tensor_copy(raw, ps)
                    else:
                        nc.scalar.copy(raw, ps)
                    # square on ScalarE, NOT GpSimd: a second GpSimd op type
                    # would alternate Q7 libraries with PartitionBroadcast and
                    # each switch costs ~8us of library reload (HAM) stall.
                    # (Not DVE either: the ssq matmul in the PE stream then
                    # waits on DVE's colsum-add backlog -- measured +20us.)
                    sq = pw.tile([128, 512], BF16, tag="sq")
                    nc.scalar.activation(out=sq, in_=ps, func=AF.Square,
                                         bias=zero128[:, :])
                    ssq = ps_ssq.tile([1, 512], F32, tag="row")
                    nc.tensor.matmul(ssq, ones_col, sq, start=True, stop=True)
                    lnr = pwrow.tile([1, 512], F32, tag="lnr")
                    if is_q:
                        # bias HD*eps with scale 1: rec = (ssq+HD*eps)^-1/2
                        # = rsqrt(mean+eps)/sqrt(HD) -- folds the score scale.
                        nc.scalar.activation(out=lnr, in_=ssq, func=AF.Ln,
                                             scale=1.0, bias=epsq[0:1, :])
                    else:
                        nc.scalar.activation(out=lnr, in_=ssq, func=AF.Ln,
                                             scale=1.0 / HD, bias=epsk[0:1, :])
                    rec = pwrow.tile([1, 512], F32, tag="rec")
                    nc.scalar.activation(out=rec, in_=lnr, func=AF.Exp,
                                         scale=-0.5, bias=zero128[0:1, :])
                    bc = pwrow.tile([128, 512], F32, tag="bc")
                    nc.gpsimd.partition_broadcast(bc, rec)
                    yield
                    # rope: out = (raw*cos + swap64(raw)*sin) * bc
                    # (gamma and the sign are folded into the host tables)
                    m1 = pw.tile([128, 512], BF16, tag="m1")
                    nc.vector.tensor_mul(m1, raw, cos_t[:, cs])
                    swp = pw.tile([128, 512], BF16, tag="swp")
                    nc.vector.tensor_copy(swp[0:64, :], raw[64:128, :])
                    nc.vector.tensor_copy(swp[64:128, :], raw[0:64, :])
                    m2 = pw.tile([128, 512], BF16, tag="m2")
                    nc.vector.tensor_mul(m2, swp, sin_t[:, cs])
                    nc.vector.tensor_add(m1, m1, m2)
                    nc.vector.tensor_mul(out_T[:, h, cs], m1, bc)
                    yield

            # all 4 K heads round-robin one chunk at a time: ~13.8us of PE
            # work per xt chunk arrival (~5.3us) keeps the PE fed from ~4us.
            _roundrobin([qk_proj_steps(kv, wk_d, kT, cosk, sink, is_q=False,
                                       wsl=wsl_k[kv]) for kv in range(NKV)],
                        steps_per_turn=3)
            ktab_cm.__exit__(None, None, None)

            # V projection
            for tt in range(NTT):
                v_ps = ps_mm.tile([128, 512], F32, tag="mm")
                ts_ = slice(tt * 128, (tt + 1) * 128)
                for d in range(ND):
                    nc.tensor.matmul(v_ps, xt[:, d, ts_], wv_sb[:, d, :],
                                     start=(d == 0), stop=(d == ND - 1))
                nc.scalar.copy(v_sb[:, tt, :], v_ps)

            # q tables, q slabs & attention work pools open after ktab/wvp
            # closed (reuse their space; SBUF is tight during K)
            qtab = phA.enter_context(tc.tile_pool(name="qtab", bufs=1))
            cosq = qtab.tile([128, T], BF16, tag="cosq")
            sinq = qtab.tile([128, T], BF16, tag="sinq")
            nc.sync.dma_start(out=cosq[:], in_=cosq_d[:])
            nc.sync.dma_start(out=sinq[:], in_=sinq_d[:])
            wslab_p = phA.enter_context(tc.tile_pool(name="wslab", bufs=2))
            ap_p = phA.enter_context(tc.tile_pool(name="apw", bufs=4))
            acc_p = phA.enter_context(tc.tile_pool(name="acc", bufs=2))
            accrow = phA.enter_context(tc.tile_pool(name="accrow", bufs=1))
            ps_den = phA.enter_context(tc.tile_pool(name="ps_den", bufs=1, space="PSUM"))
            ps_s = phA.enter_context(tc.tile_pool(name="ps_s", bufs=2, space="PSUM"))
            ps_o = phA.enter_context(tc.tile_pool(name="ps_o", bufs=2, space="PSUM"))

            def attn_steps_A(h):
                """Attention over tq 0:1024 as two 512 sub-chunks (~35 yields).

                Emission order per step is s(tk+1) then o(tk) so the PE always
                has the next score matmul in hand while ScalarE exps tile tk.
                512-wide PSUM tiles keep phase-A PSUM within 8 banks."""
                kv = h // 2
                for sub in range(2):
                    cs = slice(sub * 512, (sub + 1) * 512)
                    o_ps = ps_o.tile([128, 512], F32, tag="o")
                    colsum = acc_p.tile([128, 512], BF16, tag="cs")

                    def emit_s(tk):
                        ks = slice(tk * 128, (tk + 1) * 128)
                        s_ps = ps_s.tile([128, 512], F32, tag="s")
                        nc.tensor.matmul(s_ps, kT[:, kv, ks], qT[:, h, cs],
                                         start=True, stop=True)
                        p_bf = ap_p.tile([128, 512], BF16, tag="p")
                        nc.scalar.activation(out=p_bf, in_=s_ps, func=AF.Exp,
                                             bias=zero128[:, :])
                        if tk == 0:
                            nc.vector.tensor_copy(colsum, p_bf)
                        else:
                            nc.vector.tensor_add(colsum, colsum, p_bf)
                        return p_bf

                    def emit_o(tk, p_bf):
                        vt = v_sb[:, tk, kv * HD:(kv + 1) * HD]
                        nc.tensor.matmul(o_ps, vt, p_bf,
                                         start=(tk == 0), stop=(tk == NTT - 1))

                    p_prev = emit_s(0)
                    yield
                    for tk in range(1, NTT):
                        p_cur = emit_s(tk)
                        emit_o(tk - 1, p_prev)
                        p_prev = p_cur
                        yield
                    emit_o(NTT - 1, p_prev)
                    # normalize off the PE's critical path
                    oTun = acc_p.tile([128, 512], BF16, tag="ou")
                    if sub == 0:
                        nc.vector.tensor_copy(oTun, o_ps)
                    else:
                        nc.scalar.copy(oTun, o_ps)
                    den = ps_den.tile([1, 512], F32, tag="dn")
                    nc.tensor.matmul(den, ones_col, colsum, start=True,
                                     stop=True)
                    recr = accrow.tile([1, 512], F32, tag="rr")
                    nc.vector.reciprocal_approx_fast(out=recr, in_=den)
                    bc = accrow.tile([128, 512], F32, tag="bc")
                    nc.gpsimd.partition_broadcast(bc, recr)
                    nc.vector.tensor_mul(oT0[:, h, cs], oTun, bc)
                    yield

            # Q0 plain, then attention chunk 0 of head h rides with the
            # projection of head h+1 (Q(h+1) finishes its first column chunk
            # well before attn(h+1) reads it); only attn(7) runs undiluted.
            _drain(qk_proj_steps(0, wq_d, qT, cosq, sinq, is_q=True))
            for h in range(NQH):
                att = attn_steps_A(h)
                fil = (qk_proj_steps(h + 1, wq_d, qT, cosq, sinq, is_q=True)
                       if h < NQH - 1 else None)
                _interleave(att, fil, 12.0 / 35.0)

        # ============ phase B: attention chunk 1 + out-projection ===========
        TQC = 1024
        with ExitStack() as phB:
            wop = phB.enter_context(tc.tile_pool(name="wop", bufs=1))
            wo_sb = wop.tile([128, NQH, D], BF16, tag="wo")
            # per-head slices: the first outproj matmul (qh=0) only waits
            # ~1.3us after phase A's pools close, not the full 4MB transfer
            for qh in range(NQH):
                nc.sync.dma_start(out=wo_sb[:, qh, :], in_=wo_d[:, qh, :])
            o1p = phB.enter_context(tc.tile_pool(name="o1p", bufs=1))
            oT1 = o1p.tile([128, NQH, TQC], BF16, tag="oT1")
            ysb_p = phB.enter_context(tc.tile_pool(name="ysb", bufs=4))

            def outproj_steps(tt, ps_pool, cp0, cp1):
                """Out-projection of one 128-row tile (~7 yields).  512-col
                psum pairs share each LDWEIGHTS; copies go to the engine the
                caller knows is idle (DVE mid-attention, +ScalarE in the
                tail)."""
                src = oT0 if tt < 8 else oT1
                ts_ = slice((tt % 8) * 128, (tt % 8) * 128 + 128)
                y_sb = ysb_p.tile([128, D], F32, tag="ysb")
                for np_ in range(2):
                    ns0 = slice(np_ * 1024, np_ * 1024 + 512)
                    ns1 = slice(np_ * 1024 + 512, (np_ + 1) * 1024)
                    y_ps0 = ps_pool.tile([128, 512], F32, tag="yp")
                    y_ps1 = ps_pool.tile([128, 512], F32, tag="yp")
                    for qh in range(NQH):
                        nc.tensor.matmul(y_ps0, src[:, qh, ts_],
                                         wo_sb[:, qh, ns0],
                                         start=(qh == 0), stop=(qh == NQH - 1))
                        nc.tensor.matmul(y_ps1, src[:, qh, ts_],
                                         wo_sb[:, qh, ns1],
                                         start=(qh == 0), stop=(qh == NQH - 1))
                        if qh == 3:
                            yield
                    cp0(y_sb[:, ns0], y_ps0)
                    cp1(y_sb[:, ns1], y_ps1)
                    rs = slice(tt * 128, (tt + 1) * 128)
                    nc.sync.dma_start(out=y_d[rs, np_ * 1024:(np_ + 1) * 1024],
                                      in_=y_sb[:, np_ * 1024:(np_ + 1) * 1024])
                    yield

            with ExitStack() as phB1:
                bp = phB1.enter_context(tc.tile_pool(name="bpw", bufs=4))
                bacc_p = phB1.enter_context(tc.tile_pool(name="bacc", bufs=2))
                ps_s2 = phB1.enter_context(
                    tc.tile_pool(name="ps_s2", bufs=2, space="PSUM"))
                ps_o2 = phB1.enter_context(
                    tc.tile_pool(name="ps_o2", bufs=1, space="PSUM"))
                ps_y = phB1.enter_context(
                    tc.tile_pool(name="ps_y", bufs=2, space="PSUM"))

                def attn_steps_B(h):
                    """Attention over tq 1024:2048, 1024 wide (17 yields)."""
                    kv = h // 2
                    csA = slice(TQC, TQC + 512)
                    csB = slice(TQC + 512, 2 * TQC)
                    o_ps = ps_o2.tile([128, TQC], F32, tag="o")
                    colsum = bacc_p.tile([128, TQC], BF16, tag="cs")

                    def emit_s(tk):
                        ks = slice(tk * 128, (tk + 1) * 128)
                        s_ps = ps_s2.tile([128, TQC], F32, tag="s")
                        nc.tensor.matmul(s_ps[:, 0:512], kT[:, kv, ks],
                                         qT[:, h, csA], start=True, stop=True)
                        nc.tensor.matmul(s_ps[:, 512:TQC], kT[:, kv, ks],
                                         qT[:, h, csB], start=True, stop=True)
                        p_bf = bp.tile([128, TQC], BF16, tag="p")
                        nc.scalar.activation(out=p_bf, in_=s_ps, func=AF.Exp,
                                             bias=zero128[:, :])
                        if tk == 0:
                            nc.vector.tensor_copy(colsum, p_bf)
                        else:
                            nc.vector.tensor_add(colsum, colsum, p_bf)
                        return p_bf

                    def emit_o(tk, p_bf):
                        vt = v_sb[:, tk, kv * HD:(kv + 1) * HD]
                        st, sp = (tk == 0), (tk == NTT - 1)
                        nc.tensor.matmul(o_ps[:, 0:512], vt, p_bf[:, 0:512],
                                         start=st, stop=sp)
                        nc.tensor.matmul(o_ps[:, 512:TQC], vt,
                                         p_bf[:, 512:TQC], start=st, stop=sp)

                    p_prev = emit_s(0)
                    yield
                    for tk in range(1, NTT):
                        p_cur = emit_s(tk)
                        emit_o(tk - 1, p_prev)
                        p_prev = p_cur
                        yield
                    emit_o(NTT - 1, p_prev)
                    oTun = bacc_p.tile([128, TQC], BF16, tag="ou")
                    nc.vector.tensor_copy(oTun, o_ps)
                    # denominator rows borrow an s-pool tile (row 0 only)
                    den = ps_s2.tile([128, TQC], F32, tag="s")
                    nc.tensor.matmul(den[0:1, 0:512], ones_col,
                                     colsum[:, 0:512], start=True, stop=True)
                    nc.tensor.matmul(den[0:1, 512:TQC], ones_col,
                                     colsum[:, 512:TQC], start=True, stop=True)
                    recr = bacc_p.tile([1, TQC], F32, tag="rr")
                    nc.vector.reciprocal_approx_fast(out=recr,
                                                     in_=den[0:1, :])
                    bc = bacc_p.tile([128, TQC], F32, tag="bc")
                    nc.gpsimd.partition_broadcast(bc, recr)
                    nc.vector.tensor_mul(oT1[:, h, :], oTun, bc)
                    yield

                for h in range(NQH):
                    att = attn_steps_B(h)
                    op = outproj_steps(h, ps_y, nc.vector.tensor_copy,
                                       nc.vector.tensor_copy)
                    _interleave(att, op, 7.0 / 17.0)

            # tail: attention pools closed; 4-buffer psum pair rotation
            ps_y2 = phB.enter_context(
                tc.tile_pool(name="ps_y2", bufs=4, space="PSUM"))
            for tt in range(8, NTT):
                _drain(outproj_steps(tt, ps_y2, nc.vector.tensor_copy,
                                     nc.scalar.copy))

    nc.compile()
    return nc


def _get_module():
    if "nc" not in _CACHE:
        _CACHE["nc"] = build_module()
    return _CACHE["nc"]


def _pack_inputs(x, cos, sin, Wq, Wk, Wv, Wo, q_gamma, k_gamma):
    """Host-side prep: per-core input dicts with bf16 packed layouts."""
    bf16 = ml_dtypes.bfloat16
    perm = np.concatenate([np.arange(0, HD, 2), np.arange(1, HD, 2)])  # [128]
    partner = np.concatenate([perm[64:], perm[:64]])                   # gamma idx for sin term
    sign = np.concatenate([-np.ones(64), np.ones(64)]).astype(np.float32)

    cosT = np.ascontiguousarray(cos.T)  # [128, T]
    sinT = np.ascontiguousarray(sin.T)

    def tables(gamma):
        c = (cosT[perm] * gamma[perm][:, None]).astype(bf16)
        s = (sinT[perm] * sign[:, None] * gamma[partner][:, None]).astype(bf16)
        return np.ascontiguousarray(c), np.ascontiguousarray(s)

    cosq, sinq = tables(q_gamma.astype(np.float32))
    cosk, sink = tables(k_gamma.astype(np.float32))

    per_hg = []
    for hg in range(2):
        qh = slice(hg * NQH * HD, (hg + 1) * NQH * HD)
        kh = slice(hg * NKV * HD, (hg + 1) * NKV * HD)
        wq = Wq[:, qh].reshape(ND, 128, NQH, HD)[..., perm]
        wq = np.ascontiguousarray(wq.transpose(2, 1, 0, 3)).astype(bf16)
        wk = Wk[:, kh].reshape(ND, 128, NKV, HD)[..., perm]
        wk = np.ascontiguousarray(wk.transpose(2, 1, 0, 3)).astype(bf16)
        wv = Wv[:, kh].reshape(ND, 128, NKV * HD)
        wv = np.ascontiguousarray(wv.transpose(1, 0, 2)).astype(bf16)
        wo = Wo[hg * NQH * HD:(hg + 1) * NQH * HD, :].reshape(NQH, 128, D)
        wo = np.ascontiguousarray(wo.transpose(1, 0, 2)).astype(bf16)
        per_hg.append(dict(wq=wq, wk=wk, wv=wv, wo=wo))

    in_maps = []
    for b in range(4):
        xt = x[b].T.reshape(ND, 128, T).transpose(1, 0, 2)      # [128, ND, T]
        xt = xt.reshape(128, ND, NCH, 512).transpose(2, 0, 1, 3)  # chunk-major
        xt = np.ascontiguousarray(xt).astype(bf16)
        for hg in range(2):
            m = dict(xt=xt, cosq=cosq, sinq=sinq, cosk=cosk, sink=sink,
                     **per_hg[hg])
            in_maps.append(m)
    return in_maps


def kernel(x, cos, sin, Wq, Wk, Wv, Wo, q_gamma, k_gamma, **run_kwargs):
    global LAST_RESULTS
    args = [np.asarray(a, dtype=np.float32)
            for a in (x, cos, sin, Wq, Wk, Wv, Wo, q_gamma, k_gamma)]
    nc = _get_module()
    in_maps = _pack_inputs(*args)
    res = run_bass_kernel_spmd(nc, in_maps, core_ids=list(range(8)), **run_kwargs)
    LAST_RESULTS = res
    y = np.empty((4, T, D), dtype=np.float32)
    for b in range(4):
        y[b] = np.asarray(res.results[2 * b]["y"]) + np.asarray(res.results[2 * b + 1]["y"])
    return y


# revision 37
# speedup vs baseline: 1.1901x; 1.1901x over previous
"""Trainium2 Bass kernel for GQA attention block (nn_Attention_6219112644965).

Reference computation (per batch b):
  q = rope(rmsnorm(x @ Wq, q_gamma), cos, sin)   # 16 heads x 128
  k = rope(rmsnorm(x @ Wk, k_gamma), cos, sin)   # 8 kv heads x 128
  v = x @ Wv
  o = softmax(q k^T / sqrt(128)) v               # GQA: q head h uses kv head h//2
  y = o @ Wo
Sharding: 8 cores = 4 batches x 2 head-groups; each core emits a partial
y (its 8 heads' contribution); host sums the two partials per batch.

Everything is computed head-dim-major (q^T, k^T: [HD=128 partitions, T free])
so no transposes are needed anywhere: projections use the weight slab as
stationary lhsT over x^T, scores/out accumulate transposed, and the final
y = (o^T)^T Wo.  Partition-axis reductions (rmsnorm sum of squares, softmax
denominator) are ones-matmuls ([1,512] rows, ~0.3us each) -- measured much
faster end-to-end than GpSimd partition_all_reduce (~4us + library reloads).

Schedule (the point of this version): one merged front phase emits
K-projections, V, then software-pipelines attention chunk 0 of head h with the
projection of head h+2 at ~1us instruction granularity, so ScalarE's exp
stream (the attention pacer: ~0.56us/512-tile vs ~0.43us of PE work) hides
under projection matmuls instead of stalling the in-order PE.  xt is loaded
column-chunk-major so the first projection chain starts ~6us in (one 2.1MB
chunk) instead of waiting for the full 8.4MB.  The back phase interleaves
attention chunk 1 with the chunk-0 out-projection rows, and the tail
out-projection gets its own 4-buffer PSUM pool so PSUM->SBUF copies never gate
the matmul stream.  softmax max-subtraction is skipped (scores are O(5) for
rmsnorm-ed q,k; fp32 exp is exact there); rmsnorm scale and rope tables are
host-folded (head-dim permuted so the rope pair shuffle is a 64-partition
half swap; gamma and the rope sign live in the cos/sin tables; the rmsnorm
row scale commutes with rope and is applied once at the end).
"""
import sys

sys.path.insert(0, "/opt/trn_rl_repo")

from contextlib import ExitStack

import ml_dtypes
import numpy as np

import bass_rust
import concourse.bass as bass
import concourse.mybir as mybir
import concourse.tile as tile
from concourse import bacc, bass_isa, hw_specs
from concourse.bass_utils import run_bass_kernel_spmd

F32 = mybir.dt.float32
BF16 = mybir.dt.bfloat16
AF = mybir.ActivationFunctionType
RADD = bass_isa.ReduceOp.add

T = 2048          # sequence length
D = 2048          # model dim
HD = 128          # head dim
NQH = 8           # q heads per core
NKV = 4           # kv heads per core
ND = D // 128     # 16 d-tiles
NTT = T // 128    # 16 t-tiles
NCH = T // 512    # 4 column chunks
EPS = 1e-6

_CACHE = {}
LAST_RESULTS = None


class _Bacc(bacc.Bacc):
    """Bacc with Exp pinned to the natural_log_exp_and_others ACT table set.

    The default static func->set assignment maps Exp to `exp_and_others`
    and Ln to `natural_log_exp_and_others`; a kernel alternating Ln and Exp
    then reloads the ACT tables (~2.7us) on every transition.  Hiding `exp`
    from the other sets makes both resolve to the shared set, so the table
    is loaded once for the whole kernel.
    """

    def insert_act_table_loads(self):
        has_activation = any(
            isinstance(i, mybir.InstActivation)
            for b in self.main_func.blocks
            for i in b.instructions
        )
        if not has_activation:
            return
        tables = []
        for name, funcs in hw_specs.get_activation_tables(self.m.arch).items():
            if name != "natural_log_exp_and_others":
                funcs = funcs - {AF.Exp}
            tables.append((name, funcs))
        bass_rust.insert_act_table_loads(self, tables)


def _drain(g):
    for _ in g:
        pass


def _roundrobin(gens, steps_per_turn):
    """Advance each generator steps_per_turn steps in rotation until all are
    exhausted.  Used to let several K-projection heads consume the chunk-major
    xt stream at its DMA arrival rate."""
    alive = list(gens)
    while alive:
        for g in list(alive):
            for _ in range(steps_per_turn):
                try:
                    next(g)
                except StopIteration:
                    alive.remove(g)
                    break


def _interleave(main, filler, fill_per_main):
    """Drive `main`; after each of its steps emit ~fill_per_main steps of
    `filler`.  Threads PE-dense filler work between exp-gated attention steps
    so the in-order PE stream never waits on ScalarE."""
    credit = 0.0
    alive = filler is not None
    for _ in main:
        credit += fill_per_main
        while credit >= 1.0 and alive:
            try:
                next(filler)
            except StopIteration:
                alive = False
            credit -= 1.0
    while alive:
        try:
            next(filler)
        except StopIteration:
            alive = False


def build_module():
    """Build the per-core Bass program (identical on all 8 cores)."""
    nc = _Bacc("TRN2", target_bir_lowering=False, debug=False)

    # ---- DRAM I/O (host-packed so every DMA is contiguous) ----
    xt_d = nc.dram_tensor("xt", [NCH, 128, ND, 512], BF16, kind="ExternalInput")
    wq_d = nc.dram_tensor("wq", [NQH, 128, ND, HD], BF16, kind="ExternalInput")
    wk_d = nc.dram_tensor("wk", [NKV, 128, ND, HD], BF16, kind="ExternalInput")
    wv_d = nc.dram_tensor("wv", [128, ND, NKV * HD], BF16, kind="ExternalInput")
    wo_d = nc.dram_tensor("wo", [128, NQH, D], BF16, kind="ExternalInput")
    cosq_d = nc.dram_tensor("cosq", [128, T], BF16, kind="ExternalInput")
    sinq_d = nc.dram_tensor("sinq", [128, T], BF16, kind="ExternalInput")
    cosk_d = nc.dram_tensor("cosk", [128, T], BF16, kind="ExternalInput")
    sink_d = nc.dram_tensor("sink", [128, T], BF16, kind="ExternalInput")
    y_d = nc.dram_tensor("y", [T, D], F32, kind="ExternalOutput")

    with tile.TileContext(nc) as tc, ExitStack() as top:
        persist = top.enter_context(tc.tile_pool(name="persist", bufs=1))
        qT = persist.tile([128, NQH, T], BF16, tag="qT")     # q^T, rope+norm done
        kT = persist.tile([128, NKV, T], BF16, tag="kT")     # k^T, rope+norm done
        v_sb = persist.tile([128, NTT, NKV * HD], BF16, tag="v")  # v natural
        oT0 = persist.tile([128, NQH, 1024], BF16, tag="oT0")  # attn out, tq 0:1024
        ones_col = persist.tile([128, 1], BF16, tag="ones")
        nc.vector.memset(ones_col, 1.0)
        zero128 = persist.tile([128, 1], F32, tag="zero128")
        nc.vector.memset(zero128, 0.0)
        epsq = persist.tile([128, 1], F32, tag="epsq")
        nc.vector.memset(epsq, float(HD * EPS))
        epsk = persist.tile([128, 1], F32, tag="epsk")
        nc.vector.memset(epsk, float(EPS))

        # ================= phase A: projections + attention chunk 0 =========
        with ExitStack() as phA:
            xtp = phA.enter_context(tc.tile_pool(name="xtp", bufs=1))
            xt = xtp.tile([128, ND, T], BF16, tag="xt")
            pw = phA.enter_context(tc.tile_pool(name="pw", bufs=2))
            pwrow = phA.enter_context(tc.tile_pool(name="pwrow", bufs=1))
            ps_mm = phA.enter_context(tc.tile_pool(name="ps_mm", bufs=2, space="PSUM"))
            ps_ssq = phA.enter_context(tc.tile_pool(name="ps_ssq", bufs=1, space="PSUM"))
            wvp = phA.enter_context(tc.tile_pool(name="wvp", bufs=1))
            wv_sb = wvp.tile([128, ND, NKV * HD], BF16, tag="wv")
            # ktab scoped: its 24K frees right after K for the q-table/slab
            # pools (whose DMAs then fire at ~112us, well before Q0 needs
            # them, instead of waiting for a later pool-close event)
            ktab_cm = tc.tile_pool(name="ktab", bufs=1)
            ktab = ktab_cm.__enter__()
            cosk = ktab.tile([128, T], BF16, tag="cosk")
            sink = ktab.tile([128, T], BF16, tag="sink")
            wsl_k = ktab.tile([NKV, 128, ND, HD], BF16, tag="wslk")

            # DMA order IS the arrival order (one striped queue): k-head
            # slabs and the first xt chunk lead so the K matmul round-robin
            # starts ~4us in and then tracks the xt chunk arrivals.
            nc.sync.dma_start(out=wsl_k[0], in_=wk_d[0])
            nc.sync.dma_start(out=wsl_k[1], in_=wk_d[1])
            nc.sync.dma_start(out=xt[:, 0:8, 0:512], in_=xt_d[0][:, 0:8, :])
            nc.sync.dma_start(out=xt[:, 8:ND, 0:512], in_=xt_d[0][:, 8:ND, :])
            nc.sync.dma_start(out=cosk[:], in_=cosk_d[:])
            nc.sync.dma_start(out=sink[:], in_=sink_d[:])
            nc.sync.dma_start(out=wsl_k[2], in_=wk_d[2])
            nc.sync.dma_start(out=wsl_k[3], in_=wk_d[3])
            for ch in range(1, NCH):
                nc.sync.dma_start(out=xt[:, :, ch * 512:(ch + 1) * 512],
                                  in_=xt_d[ch])
            nc.sync.dma_start(out=wv_sb[:], in_=wv_d[:])

            def qk_proj_steps(h, w_dram, out_T, cos_t, sin_t, is_q, wsl=None):
                """One head's projection + rmsnorm + rope (3 yields/chunk).

                Per 512-chunk: 16 chained matmuls, then sum-of-squares over
                the head dim via ones-matmul, rsqrt via ln/exp on ScalarE
                (one ACT table set), GpSimd broadcast, rope muls + final
                normalize on DVE."""
                if wsl is None:
                    wsl = wslab_p.tile([128, ND, HD], BF16, tag="wsl")
                    nc.sync.dma_start(out=wsl[:], in_=w_dram[h])
                for c in range(NCH):
                    cs = slice(c * 512, (c + 1) * 512)
                    ps = ps_mm.tile([128, 512], F32, tag="mm")
                    for d in range(ND):
                        nc.tensor.matmul(ps, wsl[:, d, :], xt[:, d, cs],
                                         start=(d == 0), stop=(d == ND - 1))
                        if d == 7:
                            yield
                    raw = pw.tile([128, 512], BF16, tag="raw")
                    # alternate the copy between DVE and ScalarE: in the
                    # interleaved window both run near 85% -- pinning it to
                    # either one makes that engine the pacer
                    if c % 2 == 0:
                        nc.vector.tensor_copy(raw, ps)
                    else:
                        nc.scalar.copy(raw, ps)
                    # square on ScalarE, NOT GpSimd: a second GpSimd op type
                    # would alternate Q7 libraries with PartitionBroadcast and
                    # each switch costs ~8us of library reload (HAM) stall.
                    # (Not DVE either: the ssq matmul in the PE stream then
                    # waits on DVE's colsum-add backlog -- measured +20us.)
                    sq = pw.tile([128, 512], BF16, tag="sq")
                    nc.scalar.activation(out=sq, in_=ps, func=AF.Square,
                                         bias=zero128[:, :])
                    ssq = ps_ssq.tile([1, 512], F32, tag="row")
                    nc.tensor.matmul(ssq, ones_col, sq, start=True, stop=True)
                    lnr = pwrow.tile([1, 512], F32, tag="lnr")
                    if is_q:
                        # bias HD*eps with scale 1: rec = (ssq+HD*eps)^-1/2
                        # = rsqrt(mean+eps)/sqrt(HD) -- folds the score scale.
                        nc.scalar.activation(out=lnr, in_=ssq, func=AF.Ln,
                                             scale=1.0, bias=epsq[0:1, :])
                    else:
                        nc.scalar.activation(out=lnr, in_=ssq, func=AF.Ln,
                                             scale=1.0 / HD, bias=epsk[0:1, :])
                    rec = pwrow.tile([1, 512], F32, tag="rec")
                    nc.scalar.activation(out=rec, in_=lnr, func=AF.Exp,
                                         scale=-0.5, bias=zero128[0:1, :])
                    bc = pwrow.tile([128, 512], F32, tag="bc")
                    nc.gpsimd.partition_broadcast(bc, rec)
                    yield
                    # rope: out = (raw*cos + swap64(raw)*sin) * bc
                    # (gamma and the sign are folded into the host tables)
                    m1 = pw.tile([128, 512], BF16, tag="m1")
                    nc.vector.tensor_mul(m1, raw, cos_t[:, cs])
                    swp = pw.tile([128, 512], BF16, tag="swp")
                    nc.vector.tensor_copy(swp[0:64, :], raw[64:128, :])
                    nc.vector.tensor_copy(swp[64:128, :], raw[0:64, :])
                    m2 = pw.tile([128, 512], BF16, tag="m2")
                    nc.vector.tensor_mul(m2, swp, sin_t[:, cs])
                    nc.vector.tensor_add(m1, m1, m2)
                    nc.vector.tensor_mul(out_T[:, h, cs], m1, bc)
                    yield

            # all 4 K heads round-robin one chunk at a time: ~13.8us of PE
            # work per xt chunk arrival (~5.3us) keeps the PE fed from ~4us.
            _roundrobin([qk_proj_steps(kv, wk_d, kT, cosk, sink, is_q=False,
                                       wsl=wsl_k[kv]) for kv in range(NKV)],
                        steps_per_turn=3)
            ktab_cm.__exit__(None, None, None)

            # V projection
            for tt in range(NTT):
                v_ps = ps_mm.tile([128, 512], F32, tag="mm")
                ts_ = slice(tt * 128, (tt + 1) * 128)
                for d in range(ND):
                    nc.tensor.matmul(v_ps, xt[:, d, ts_], wv_sb[:, d, :],
                                     start=(d == 0), stop=(d == ND - 1))
                nc.scalar.copy(v_sb[:, tt, :], v_ps)

            # q tables, q slabs & attention work pools open after ktab/wvp
            # closed (reuse their space; SBUF is tight during K)
            qtab = phA.enter_context(tc.tile_pool(name="qtab", bufs=1))
            cosq = qtab.tile([128, T], BF16, tag="cosq")
            sinq = qtab.tile([128, T], BF16, tag="sinq")
            nc.sync.dma_start(out=cosq[:], in_=cosq_d[:])
            nc.sync.dma_start(out=sinq[:], in_=sinq_d[:])
            wslab_p = phA.enter_context(tc.tile_pool(name="wslab", bufs=2))
            ap_p = phA.enter_context(tc.tile_pool(name="apw", bufs=4))
            acc_p = phA.enter_context(tc.tile_pool(name="acc", bufs=2))
            accrow = phA.enter_context(tc.tile_pool(name="accrow", bufs=1))
            ps_den = phA.enter_context(tc.tile_pool(name="ps_den", bufs=1, space="PSUM"))
            ps_s = phA.enter_context(tc.tile_pool(name="ps_s", bufs=2, space="PSUM"))
            ps_o = phA.enter_context(tc.tile_pool(name="ps_o", bufs=2, space="PSUM"))

            def attn_steps_A(h):
                """Attention over tq 0:1024 as two 512 sub-chunks (~35 yields).

                Emission order per step is s(tk+1) then o(tk) so the PE always
                has the next score matmul in hand while ScalarE exps tile tk.
                512-wide PSUM tiles keep phase-A PSUM within 8 banks."""
                kv = h // 2
                for sub in range(2):
                    cs = slice(sub * 512, (sub + 1) * 512)
                    o_ps = ps_o.tile([128, 512], F32, tag="o")
                    colsum = acc_p.tile([128, 512], BF16, tag="cs")

                    def emit_s(tk):
                        ks = slice(tk * 128, (tk + 1) * 128)
                        s_ps = ps_s.tile([128, 512], F32, tag="s")
                        nc.tensor.matmul(s_ps, kT[:, kv, ks], qT[:, h, cs],
                                         start=True, stop=True)
                        p_bf = ap_p.tile([128, 512], BF16, tag="p")
                        nc.scalar.activation(out=p_bf, in_=s_ps, func=AF.Exp,
                                             bias=zero128[:, :])
                        if tk == 0:
                            nc.vector.tensor_copy(colsum, p_bf)
                        else:
                            nc.vector.tensor_add(colsum, colsum, p_bf)
                        return p_bf

                    def emit_o(tk, p_bf):
                        vt = v_sb[:, tk, kv * HD:(kv + 1) * HD]
                        nc.tensor.matmul(o_ps, vt, p_bf,
                                         start=(tk == 0), stop=(tk == NTT - 1))

                    p_prev = emit_s(0)
                    yield
                    for tk in range(1, NTT):
                        p_cur = emit_s(tk)
                        emit_o(tk - 1, p_prev)
                        p_prev = p_cur
                        yield
                    emit_o(NTT - 1, p_prev)
                    # normalize off the PE's critical path
                    oTun = acc_p.tile([128, 512], BF16, tag="ou")
                    if sub == 0:
                        nc.vector.tensor_copy(oTun, o_ps)
                    else:
                        nc.scalar.copy(oTun, o_ps)
                    den = ps_den.tile([1, 512], F32, tag="dn")
                    nc.tensor.matmul(den, ones_col, colsum, start=True,
                                     stop=True)
                    recr = accrow.tile([1, 512], F32, tag="rr")
                    nc.vector.reciprocal_approx_fast(out=recr, in_=den)
                    bc = accrow.tile([128, 512], F32, tag="bc")
                    nc.gpsimd.partition_broadcast(bc, recr)
                    nc.vector.tensor_mul(oT0[:, h, cs], oTun, bc)
                    yield

            # Q0 plain, then attention chunk 0 of head h rides with the
            # projection of head h+1 (Q(h+1) finishes its first column chunk
            # well before attn(h+1) reads it); only attn(7) runs undiluted.
            _drain(qk_proj_steps(0, wq_d, qT, cosq, sinq, is_q=True))
            for h in range(NQH):
                att = attn_steps_A(h)
                fil = (qk_proj_steps(h + 1, wq_d, qT, cosq, sinq, is_q=True)
                       if h < NQH - 1 else None)
                _interleave(att, fil, 12.0 / 35.0)

        # ============ phase B: attention chunk 1 + out-projection ===========
        TQC = 1024
        with ExitStack() as phB:
            wop = phB.enter_context(tc.tile_pool(name="wop", bufs=1))
            wo_sb = wop.tile([128, NQH, D], BF16, tag="wo")
            # per-head slices: the first outproj matmul (qh=0) only waits
            # ~1.3us after phase A's pools close, not the full 4MB transfer
            for qh in range(NQH):
                nc.sync.dma_start(out=wo_sb[:, qh, :], in_=wo_d[:, qh, :])
            o1p = phB.enter_context(tc.tile_pool(name="o1p", bufs=1))
            oT1 = o1p.tile([128, NQH, TQC], BF16, tag="oT1")
            ysb_p = phB.enter_context(tc.tile_pool(name="ysb", bufs=4))

            def outproj_steps(tt, ps_pool, cp0, cp1):
                """Out-projection of one 128-row tile (~7 yields).  512-col
                psum pairs share each LDWEIGHTS; copies go to the engine the
                caller knows is idle (DVE mid-attention, +ScalarE in the
                tail)."""
                src = oT0 if tt < 8 else oT1
                ts_ = slice((tt % 8) * 128, (tt % 8) * 128 + 128)
                y_sb = ysb_p.tile([128, D], F32, tag="ysb")
                for np_ in range(2):
                    ns0 = slice(np_ * 1024, np_ * 1024 + 512)
                    ns1 = slice(np_ * 1024 + 512, (np_ + 1) * 1024)
                    y_ps0 = ps_pool.tile([128, 512], F32, tag="yp")
                    y_ps1 = ps_pool.tile([128, 512], F32, tag="yp")
                    for qh in range(NQH):
                        nc.tensor.matmul(y_ps0, src[:, qh, ts_],
                                         wo_sb[:, qh, ns0],
                                         start=(qh == 0), stop=(qh == NQH - 1))
                        nc.tensor.matmul(y_ps1, src[:, qh, ts_],
                                         wo_sb[:, qh, ns1],
                                         start=(qh == 0), stop=(qh == NQH - 1))
                        if qh == 3:
                            yield
                    cp0(y_sb[:, ns0], y_ps0)
                    cp1(y_sb[:, ns1], y_ps1)
                    rs = slice(tt * 128, (tt + 1) * 128)
                    nc.sync.dma_start(out=y_d[rs, np_ * 1024:(np_ + 1) * 1024],
                                      in_=y_sb[:, np_ * 1024:(np_ + 1) * 1024])
                    yield

            with ExitStack() as phB1:
                bp = phB1.enter_context(tc.tile_pool(name="bpw", bufs=4))
                bacc_p = phB1.enter_context(tc.tile_pool(name="bacc", bufs=2))
                ps_s2 = phB1.enter_context(
                    tc.tile_pool(name="ps_s2", bufs=2, space="PSUM"))
                ps_o2 = phB1.enter_context(
                    tc.tile_pool(name="ps_o2", bufs=1, space="PSUM"))
                ps_y = phB1.enter_context(
                    tc.tile_pool(name="ps_y", bufs=2, space="PSUM"))

                def attn_steps_B(h):
                    """Attention over tq 1024:2048, 1024 wide (17 yields)."""
                    kv = h // 2
                    csA = slice(TQC, TQC + 512)
                    csB = slice(TQC + 512, 2 * TQC)
                    o_ps = ps_o2.tile([128, TQC], F32, tag="o")
                    colsum = bacc_p.tile([128, TQC], BF16, tag="cs")

                    def emit_s(tk):
                        ks = slice(tk * 128, (tk + 1) * 128)
                        s_ps = ps_s2.tile([128, TQC], F32, tag="s")
                        nc.tensor.matmul(s_ps[:, 0:512], kT[:, kv, ks],
                                         qT[:, h, csA], start=True, stop=True)
                        nc.tensor.matmul(s_ps[:, 512:TQC], kT[:, kv, ks],
                                         qT[:, h, csB], start=True, stop=True)
                        p_bf = bp.tile([128, TQC], BF16, tag="p")
                        nc.scalar.activation(out=p_bf, in_=s_ps, func=AF.Exp,
                                             bias=zero128[:, :])
                        if tk == 0:
                            nc.vector.tensor_copy(colsum, p_bf)
                        else:
                            nc.vector.tensor_add(colsum, colsum, p_bf)
                        return p_bf

                    def emit_o(tk, p_bf):
                        vt = v_sb[:, tk, kv * HD:(kv + 1) * HD]
                        st, sp = (tk == 0), (tk == NTT - 1)
                        nc.tensor.matmul(o_ps[:, 0:512], vt, p_bf[:, 0:512],
                                         start=st, stop=sp)
                        nc.tensor.matmul(o_ps[:, 512:TQC], vt,
                                         p_bf[:, 512:TQC], start=st, stop=sp)

                    p_prev = emit_s(0)
                    yield
                    for tk in range(1, NTT):
                        p_cur = emit_s(tk)
                        emit_o(tk - 1, p_prev)
                        p_prev = p_cur
                        yield
                    emit_o(NTT - 1, p_prev)
                    oTun = bacc_p.tile([128, TQC], BF16, tag="ou")
                    nc.vector.tensor_copy(oTun, o_ps)
                    # denominator rows borrow an s-pool tile (row 0 only)
                    den = ps_s2.tile([128, TQC], F32, tag="s")
                    nc.tensor.matmul(den[0:1, 0:512], ones_col,
                                     colsum[:, 0:512], start=True, stop=True)
                    nc.tensor.matmul(den[0:1, 512:TQC], ones_col,
                                     colsum[:, 512:TQC], start=True, stop=True)
                    recr = bacc_p.tile([1, TQC], F32, tag="rr")
                    nc.vector.reciprocal_approx_fast(out=recr,
                                                     in_=den[0:1, :])
                    bc = bacc_p.tile([128, TQC], F32, tag="bc")
                    nc.gpsimd.partition_broadcast(bc, recr)
                    nc.vector.tensor_mul(oT1[:, h, :], oTun, bc)
                    yield

                for h in range(NQH):
                    att = attn_steps_B(h)
                    op = outproj_steps(h, ps_y, nc.vector.tensor_copy,
                                       nc.vector.tensor_copy)
                    _interleave(att, op, 7.0 / 17.0)

            # tail: attention pools closed; 4-buffer psum pair rotation
            ps_y2 = phB.enter_context(
                tc.tile_pool(name="ps_y2", bufs=4, space="PSUM"))
            for tt in range(8, NTT):
                _drain(outproj_steps(tt, ps_y2, nc.vector.tensor_copy,
                                     nc.scalar.copy))

    nc.compile()
    return nc


def _get_module():
    if "nc" not in _CACHE:
        _CACHE["nc"] = build_module()
    return _CACHE["nc"]


def _pack_inputs(x, cos, sin, Wq, Wk, Wv, Wo, q_gamma, k_gamma):
    """Host-side prep: per-core input dicts with bf16 packed layouts."""
    bf16 = ml_dtypes.bfloat16
    perm = np.concatenate([np.arange(0, HD, 2), np.arange(1, HD, 2)])  # [128]
    partner = np.concatenate([perm[64:], perm[:64]])                   # gamma idx for sin term
    sign = np.concatenate([-np.ones(64), np.ones(64)]).astype(np.float32)

    cosT = np.ascontiguousarray(cos.T)  # [128, T]
    sinT = np.ascontiguousarray(sin.T)

    def tables(gamma):
        c = (cosT[perm] * gamma[perm][:, None]).astype(bf16)
        s = (sinT[perm] * sign[:, None] * gamma[partner][:, None]).astype(bf16)
        return np.ascontiguousarray(c), np.ascontiguousarray(s)

    cosq, sinq = tables(q_gamma.astype(np.float32))
    cosk, sink = tables(k_gamma.astype(np.float32))

    per_hg = []
    for hg in range(2):
        qh = slice(hg * NQH * HD, (hg + 1) * NQH * HD)
        kh = slice(hg * NKV * HD, (hg + 1) * NKV * HD)
        wq = Wq[:, qh].reshape(ND, 128, NQH, HD)[..., perm]
        wq = np.ascontiguousarray(wq.transpose(2, 1, 0, 3)).astype(bf16)
        wk = Wk[:, kh].reshape(ND, 128, NKV, HD)[..., perm]
        wk = np.ascontiguousarray(wk.transpose(2, 1, 0, 3)).astype(bf16)
        wv = Wv[:, kh].reshape(ND, 128, NKV * HD)
        wv = np.ascontiguousarray(wv.transpose(1, 0, 2)).astype(bf16)
        wo = Wo[hg * NQH * HD:(hg + 1) * NQH * HD, :].reshape(NQH, 128, D)
        wo = np.ascontiguousarray(wo.transpose(1, 0, 2)).astype(bf16)
        per_hg.append(dict(wq=wq, wk=wk, wv=wv, wo=wo))

    in_maps = []
    for b in range(4):
        xt = x[b].T.reshape(ND, 128, T).transpose(1, 0, 2)      # [128, ND, T]
        xt = xt.reshape(128, ND, NCH, 512).transpose(2, 0, 1, 3)  # chunk-major
        xt = np.ascontiguousarray(xt).astype(bf16)
        for hg in range(2):
            m = dict(xt=xt, cosq=cosq, sinq=sinq, cosk=cosk, sink=sink,
                     **per_hg[hg])
            in_maps.append(m)
    return in_maps


def kernel(x, cos, sin, Wq, Wk, Wv, Wo, q_gamma, k_gamma, **run_kwargs):
    global LAST_RESULTS
    args = [np.asarray(a, dtype=np.float32)
            for a in (x, cos, sin, Wq, Wk, Wv, Wo, q_gamma, k_gamma)]
    nc = _get_module()
    in_maps = _pack_inputs(*args)
    res = run_bass_kernel_spmd(nc, in_maps, core_ids=list(range(8)), **run_kwargs)
    LAST_RESULTS = res
    y = np.empty((4, T, D), dtype=np.float32)
    for b in range(4):
        y[b] = np.asarray(res.results[2 * b]["y"]) + np.asarray(res.results[2 * b + 1]["y"])
    return y
